# revision 1
# baseline (speedup 1.0000x reference)
"""MoE routing kernel for Trainium2 (8 NeuronCores, expert-parallel).

Sharding strategy (per the expert-parallel hint):
  - Core c owns expert c's weights (E=8 experts, 8 cores).
  - Token dispatch by top-k index happens at shard time: the host
    computes the gate (same math as the reference, on CPU jax so
    tie-breaking matches bit-for-bit), gathers each expert's routed
    tokens (capacity 768 >> max observed load), and ships core c its
    compacted token batch [D, C] plus combine-weight columns.
  - The shared expert is token-sharded: core c processes tokens
    [c*256, (c+1)*256) with the full (replicated) shared weights.
  - Unshard: shared outputs concatenate; routed outputs scatter-add
    back by token index (padded slots carry combine weight 0).

The device runs both FFN batches through one generic pipeline:
  h1/h3 psums (float32r matmuls, full-rate fp32 on the PE), fused
  swiglu on DVE/ACT, then the down-projection with the bias folded in
  as a K=1 matmul and the combine weight applied per-partition.

A dense all-on-device fallback (every core processes all tokens through
its expert, masked by gate weights computed on-device) is kept for the
(never observed) case that an expert's routed load exceeds capacity.
"""

import numpy as np
from contextlib import ExitStack

import concourse.bass as bass
import concourse.mybir as mybir
import concourse.tile as tile
from concourse import bacc
from concourse.bass_utils import run_bass_kernel_spmd

# Problem dims (hardcoded per contract)
E = 8
D = 1024
F = 1024
T = 2048          # B*S = 2*1024
P = 128
DK = D // P       # 8 k-chunks over D
FI = F // P       # 8 f-chunks over F (per g/l branch)
ALPHA = 1.702
LIMIT = 7.0
NCORES = 8
CAP = 768         # routed-token capacity per expert (max load ~592)
TSH = T // NCORES  # shared-expert tokens per core

F32 = mybir.dt.float32
F32R = mybir.dt.float32r
BF16 = mybir.dt.bfloat16
AF = mybir.ActivationFunctionType
OP = mybir.AluOpType


def _chunks(n):
    # fp32r needs moving free-dim >= 256 for full-rate matmuls; stick to
    # 512/256 chunk sizes (384 triggered pathological walrus lowering)
    out = []
    o = 0
    while o < n:
        s = min(512, n - o)
        out.append((o, s))
        o += s
    return out


# ---------------------------------------------------------------------------
# generic FFN-batch emitter: out[cap, D] = cw * (swiglu((xT@w1+b1)*(xT@w3+b3)) @ w2T + b2)
# ---------------------------------------------------------------------------

def _emit_ffn(tc, pools, pref, aps, cap, wdt):
    nc = tc.nc
    wA, w2p, apool, hpool, outp, psA, psB = pools

    # fi=0 weights first: the very first matmul needs w[fi0][dk0] + x[dk0],
    # so don't queue 3.5MB of x-columns ahead of them
    wt0 = {}
    for nm in ("w1g", "w1l", "w3g", "w3l"):
        t = wA.tile([P, DK, P], wdt, tag=nm)
        nc.sync.dma_start(t[:], aps[pref + nm][0])
        wt0[nm] = t

    # x columns resident
    xsb = []
    for dk in range(DK):
        t = pools[2].tile([P, cap], wdt, tag=f"{pref}x{dk}")
        nc.sync.dma_start(t[:], aps[pref + "xT"][dk * P:(dk + 1) * P, :])
        xsb.append(t)

    bcols = {}
    for n in ("b1g", "b1l", "b3g", "b3l"):
        t = pools[2].tile([P, FI], F32, tag=pref + n)
        nc.sync.dma_start(t[:], aps[pref + n][:])
        bcols[n] = t
    cwt = pools[2].tile([P, cap // P], F32, tag=pref + "cw")
    nc.sync.dma_start(cwt[:], aps[pref + "cw"][:])
    b2bc = pools[2].tile([P, D], F32, tag=pref + "b2bc")
    nc.sync.dma_start(b2bc[:], aps[pref + "b2bc"][:])

    # ---------- stage A ----------
    atiles = []
    for fi in range(FI):
        at = apool.tile([P, cap], wdt, tag=f"{pref}a{fi}")
        atiles.append(at)
        if fi == 0:
            wt = wt0
        else:
            wt = {}
            for nm in ("w1g", "w1l", "w3g", "w3l"):
                t = wA.tile([P, DK, P], wdt, tag=nm)
                nc.sync.dma_start(t[:], aps[pref + nm][fi])
                wt[nm] = t
        bc1g = bcols["b1g"][:, fi:fi + 1]
        bc1l = bcols["b1l"][:, fi:fi + 1]
        bc3g = bcols["b3g"][:, fi:fi + 1]
        bc3l = bcols["b3l"][:, fi:fi + 1]

        for (to, ts) in _chunks(cap):
            tsl = slice(to, to + ts)

            def hpsum(wtile, ptag):
                ps = psA.tile([P, 512], F32, tag=ptag)
                for dk in range(DK):
                    nc.tensor.matmul(
                        ps[:, :ts], (wtile[:, dk, :]),
                        (xsb[dk][:, tsl]),
                        start=(dk == 0), stop=(dk == DK - 1))
                return ps

            pg1 = hpsum(wt["w1g"], "pA")
            t1 = hpool.tile([P, 512], F32, tag="tcp")
            nc.scalar.activation(t1[:, :ts], pg1[:, :ts], AF.Identity,
                                 bias=bc1g)
            pg3 = hpsum(wt["w3g"], "pB")
            hg = hpool.tile([P, 512], F32, tag="hh")
            nc.vector.scalar_tensor_tensor(
                out=hg[:, :ts], in0=pg3[:, :ts], scalar=bc3g, in1=t1[:, :ts],
                op0=OP.add, op1=OP.mult)
            nc.vector.tensor_scalar_min(hg[:, :ts], hg[:, :ts], LIMIT)
            gs = hpool.tile([P, 512], F32, tag="gs")
            nc.scalar.activation(gs[:, :ts], hg[:, :ts], AF.Silu, scale=ALPHA)

            pl1 = hpsum(wt["w1l"], "pA")
            t2 = hpool.tile([P, 512], F32, tag="tcp")
            nc.scalar.activation(t2[:, :ts], pl1[:, :ts], AF.Identity,
                                 bias=bc1l)
            pl3 = hpsum(wt["w3l"], "pB")
            hl = hpool.tile([P, 512], F32, tag="hh")
            nc.vector.scalar_tensor_tensor(
                out=hl[:, :ts], in0=pl3[:, :ts], scalar=bc3l, in1=t2[:, :ts],
                op0=OP.add, op1=OP.mult)
            nc.vector.tensor_scalar(
                out=hl[:, :ts], in0=hl[:, :ts], scalar1=LIMIT, scalar2=-LIMIT,
                op0=OP.min, op1=OP.max)
            nc.vector.tensor_scalar(
                out=hl[:, :ts], in0=hl[:, :ts], scalar1=1.0 / ALPHA,
                scalar2=1.0 / ALPHA, op0=OP.mult, op1=OP.add)
            nc.vector.tensor_mul(atiles[fi][:, tsl], gs[:, :ts], hl[:, :ts])

    # ---------- stage B ----------
    for dch in range(D // 512):
        dsl = slice(dch * 512, (dch + 1) * 512)
        w2t = []
        for fi in range(FI):
            t = w2p.tile([P, 512], wdt, tag=f"w2t{fi}")
            nc.sync.dma_start(t[:], aps[pref + "w2T"][fi * P:(fi + 1) * P, dsl])
            w2t.append(t)
        for tp in range(cap // P):
            tsl = slice(tp * P, (tp + 1) * P)
            pB = psB.tile([P, 512], F32, tag="pB2")
            for fi in range(FI):
                nc.tensor.matmul(
                    pB[:], (atiles[fi][:, tsl]), (w2t[fi][:]),
                    start=(fi == 0), stop=(fi == FI - 1))
            ot = outp.tile([P, 512], F32, tag="ot")
            nc.vector.tensor_add(ot[:], pB[:], b2bc[:, dsl])
            nc.vector.tensor_scalar_mul(ot[:], ot[:], cwt[:, tp:tp + 1])
            nc.sync.dma_start(
                aps[pref + "out"][tp * P:(tp + 1) * P, dsl], ot[:])


def _build_sparse():
    nc = bacc.Bacc(
        "TRN2", target_bir_lowering=False, debug=False, num_devices=NCORES
    )
    aps = {}

    def inp(name, shape, dt=F32):
        aps[name] = nc.dram_tensor(name, shape, dt, kind="ExternalInput").ap()

    for pref, cap, wdt in (("r_", CAP, F32R), ("s_", TSH, F32R)):
        inp(pref + "xT", [D, cap], wdt)
        for n in ("w1g", "w1l", "w3g", "w3l"):
            inp(pref + n, [FI, P, DK, P], wdt)
        for n in ("b1g", "b1l", "b3g", "b3l"):
            inp(pref + n, [P, FI])
        inp(pref + "w2T", [F, D], wdt)
        inp(pref + "b2bc", [P, D])
        inp(pref + "cw", [P, cap // P])
        aps[pref + "out"] = nc.dram_tensor(
            pref + "out", [cap, D], F32, kind="ExternalOutput").ap()

    with tile.TileContext(nc) as tc:
        with ExitStack() as ctx:
            wA = ctx.enter_context(tc.tile_pool(name="wA", bufs=2))
            w2p = ctx.enter_context(tc.tile_pool(name="w2p", bufs=2))
            apool = ctx.enter_context(tc.tile_pool(name="apool", bufs=1))
            hpool = ctx.enter_context(tc.tile_pool(name="hpool", bufs=2))
            outp = ctx.enter_context(tc.tile_pool(name="outp", bufs=3))
            psA = ctx.enter_context(
                tc.tile_pool(name="psA", bufs=2, space="PSUM"))
            psB = ctx.enter_context(
                tc.tile_pool(name="psB", bufs=2, space="PSUM"))
            pools = (wA, w2p, apool, hpool, outp, psA, psB)
            _emit_ffn(tc, pools, "r_", aps, CAP, F32R)
            _emit_ffn(tc, pools, "s_", aps, TSH, F32R)
    nc.compile()
    return nc


# ---------------------------------------------------------------------------
# host-side prep
# ---------------------------------------------------------------------------

def _warr(w):      # [F, D] -> [FI, P, DK, P] stage-A stationary layout
    return np.ascontiguousarray(
        w.T.reshape(DK, P, FI, P).transpose(2, 1, 0, 3))


def _bcol(b):      # deinterleaved [F] -> [P, FI]
    return np.ascontiguousarray(b.reshape(FI, P).T)


def _gate(x, gate_w, gate_b):
    """Replicate the reference gate on CPU jax (bit-identical math)."""
    import jax
    import jax.numpy as jnp
    cpu = jax.devices("cpu")[0]
    with jax.default_device(cpu):
        xt = jnp.asarray(np.asarray(x, np.float32).reshape(T, D))
        logits = xt @ jnp.asarray(np.asarray(gate_w, np.float32)).T
        scores = jax.nn.softmax(logits.astype(jnp.float32), axis=-1)
        biased = scores + jnp.asarray(
            np.asarray(gate_b, np.float32)).astype(jnp.float32)
        idx = jax.lax.top_k(biased, 2)[1]
        weights = jnp.take_along_axis(scores, idx, axis=-1)
        return np.asarray(idx), np.asarray(weights)


def _prep_sparse(x, gate_w, gate_b, w1, b1, w3, b3, w2, b2,
                 sw1, sb1, sw3, sb3, sw2, sb2):
    f32 = np.float32
    xt = np.asarray(x, f32).reshape(T, D)
    xT = np.ascontiguousarray(xt.T)

    idx, wts = _gate(x, gate_w, gate_b)          # [T, 2], [T, 2]
    toks = [[] for _ in range(E)]
    cws = [[] for _ in range(E)]
    for k in range(2):
        for t in range(T):
            e = int(idx[t, k])
            toks[e].append(t)
            cws[e].append(wts[t, k])
    counts = [len(v) for v in toks]
    if max(counts) > CAP:
        return None, None, None  # fall back to dense

    sw1 = np.asarray(sw1, f32)
    sw3 = np.asarray(sw3, f32)
    sb1 = np.asarray(sb1, f32)
    sb3 = np.asarray(sb3, f32)
    shared_common = {
        "s_w1g": _warr(sw1[0::2]),
        "s_w1l": _warr(sw1[1::2]),
        "s_w3g": _warr(sw3[0::2]),
        "s_w3l": _warr(sw3[1::2]),
        "s_b1g": _bcol(sb1[0::2]), "s_b1l": _bcol(sb1[1::2]),
        "s_b3g": _bcol(sb3[0::2]), "s_b3l": _bcol(sb3[1::2]),
        "s_w2T": np.ascontiguousarray(np.asarray(sw2, f32).T),
        "s_b2bc": np.ascontiguousarray(
            np.broadcast_to(np.asarray(sb2, f32), (P, D))),
        "s_cw": np.ones((P, TSH // P), f32),
    }

    in_maps, tls, ncs = [], [], []
    for c in range(NCORES):
        nc_ = counts[c]
        tl = np.zeros(CAP, np.int64)
        tl[:nc_] = toks[c]
        cwv = np.zeros(CAP, f32)
        cwv[:nc_] = cws[c]
        tls.append(tl)
        ncs.append(nc_)
        w1c = np.asarray(w1[c], f32)
        w3c = np.asarray(w3[c], f32)
        b1c = np.asarray(b1[c], f32)
        b3c = np.asarray(b3[c], f32)
        m = {
            "r_xT": np.ascontiguousarray(xT[:, tl]),
            "r_w1g": _warr(w1c[0::2]), "r_w1l": _warr(w1c[1::2]),
            "r_w3g": _warr(w3c[0::2]), "r_w3l": _warr(w3c[1::2]),
            "r_b1g": _bcol(b1c[0::2]), "r_b1l": _bcol(b1c[1::2]),
            "r_b3g": _bcol(b3c[0::2]), "r_b3l": _bcol(b3c[1::2]),
            "r_w2T": np.ascontiguousarray(np.asarray(w2[c], f32).T),
            "r_b2bc": np.ascontiguousarray(
                np.broadcast_to(np.asarray(b2[c], f32), (P, D))),
            "r_cw": np.ascontiguousarray(cwv.reshape(CAP // P, P).T),
            "s_xT": np.ascontiguousarray(xT[:, c * TSH:(c + 1) * TSH]),
        }
        m.update(shared_common)
        in_maps.append(m)
    return in_maps, tls, ncs


_PROGS = {}


def _get_program(kind):
    if kind not in _PROGS:
        _PROGS[kind] = {"sparse": _build_sparse, "dense": _build_dense}[kind]()
    return _PROGS[kind]


def kernel(x, gate_w, gate_b, w1, b1, w3, b3, w2, b2,
           sw1, sb1, sw3, sb3, sw2, sb2, _trace=False, _results=None,
           _force_dense=False):
    kw = {}
    if _trace:
        kw = dict(trace=True, trace_cores=list(range(NCORES)))
    args = (x, gate_w, gate_b, w1, b1, w3, b3, w2, b2,
            sw1, sb1, sw3, sb3, sw2, sb2)
    if not _force_dense:
        in_maps, tls, ncs = _prep_sparse(*args)
    else:
        in_maps = None
    if in_maps is not None:
        nc = _get_program("sparse")
        res = run_bass_kernel_spmd(
            nc, in_maps, core_ids=list(range(NCORES)), **kw)
        if _results is not None:
            _results.append(res)
        out = np.empty((T, D), np.float32)
        for c in range(NCORES):
            out[c * TSH:(c + 1) * TSH] = res.results[c]["s_out"]
        for c in range(NCORES):
            n = ncs[c]
            out[tls[c][:n]] += res.results[c]["r_out"][:n]
        return out.reshape(np.asarray(x).shape).astype(np.float32)

    # dense fallback
    in_maps = _prep_dense(*args)
    nc = _get_program("dense")
    res = run_bass_kernel_spmd(nc, in_maps, core_ids=list(range(NCORES)), **kw)
    if _results is not None:
        _results.append(res)
    acc = np.zeros((T, D), np.float32)
    for c in range(NCORES):
        acc += res.results[c]["out"]
    return acc.reshape(np.asarray(x).shape).astype(np.float32)


# ---------------------------------------------------------------------------
# dense all-on-device fallback (V1): every core runs its expert over all
# tokens, masked by on-device gate weights; shared expert sharded on 2F.
# ---------------------------------------------------------------------------

TCH = 512
NTH = 2
TH = T // NTH


def _build_dense():
    nc = bacc.Bacc(
        "TRN2", target_bir_lowering=False, debug=False, num_devices=NCORES
    )
    aps = {}

    def inp(name, shape, dt=F32):
        aps[name] = nc.dram_tensor(name, shape, dt, kind="ExternalInput").ap()

    inp("xT", [D, T], F32R)
    inp("gw", [P, DK * E], F32R)
    inp("gb", [P, E])
    inp("sel", [P, E])
    for n in ("w1g", "w1l", "w3g", "w3l"):
        inp(n, [FI, P, DK, P], F32R)
    for n in ("b1g", "b1l", "b3g", "b3l"):
        inp(n, [P, FI + 1])
    inp("w2T", [F, D], F32R)
    inp("b2r", [1, D], F32R)
    for n in ("sw1g", "sw1l", "sw3g", "sw3l"):
        inp(n, [P, DK, P], F32R)
    inp("sw2T", [P, D], F32R)
    inp("sb2r", [1, D], F32R)
    inp("ones", [1, P], F32R)
    aps["out"] = nc.dram_tensor("out", [T, D], F32, kind="ExternalOutput").ap()

    with tile.TileContext(nc) as tc:
        _emit_dense(tc, aps)
    nc.compile()
    return nc


def _emit_dense(tc, aps):
    nc = tc.nc
    ctx = ExitStack()

    with ctx:
        const = ctx.enter_context(tc.tile_pool(name="const", bufs=1))

        xsb = []
        for dk in range(DK):
            t = const.tile([P, T], F32R, tag=f"x{dk}")
            nc.sync.dma_start(t[:], aps["xT"][dk * P:(dk + 1) * P, :])
            xsb.append(t)

        def load_const(name, shape, dt=F32):
            t = const.tile(shape, dt, tag=name)
            nc.sync.dma_start(t[:], aps[name][:])
            return t

        gw_sb = load_const("gw", [P, DK * E], F32R)
        gb_sb = load_const("gb", [P, E])
        sel_sb = load_const("sel", [P, E])
        bcols = {n: load_const(n, [P, FI + 1])
                 for n in ("b1g", "b1l", "b3g", "b3l")}
        b2r_sb = load_const("b2r", [1, D], F32R)
        sb2r_sb = load_const("sb2r", [1, D], F32R)
        sw2T_sb = load_const("sw2T", [P, D], F32R)
        ssw = {}
        for name in ("sw1g", "sw1l", "sw3g", "sw3l"):
            t = const.tile([P, DK, P], F32R, tag=name)
            nc.sync.dma_start(t[:], aps[name][:])
            ssw[name] = t

        ones = const.tile([1, P], F32R, tag="ones")
        nc.sync.dma_start(ones[:], aps["ones"][:])
        ident = const.tile([E, E], F32, tag="ident")
        nc.vector.memset(ident[:], 0.0)
        from concourse.masks import make_identity
        make_identity(nc, ident[:], nomemset=True)

        cw = const.tile([P, T // P], F32, tag="cw")

        # ---- gate ----
        with tc.tile_pool(name="psG", bufs=2, space="PSUM") as psG, \
             tc.tile_pool(name="gtmp", bufs=1) as gtmp:
            NC = T // P
            logits_tb = const.tile([P, NC * E], F32, tag="logits_tb")
            logitsT = gtmp.tile([E, T], F32, tag="logitsT")
            for tch in range(T // TCH):
                pg = psG.tile([E, TCH], F32, tag="pslog")
                for dk in range(DK):
                    nc.tensor.matmul(
                        pg[:],
                        (gw_sb[:, dk * E:(dk + 1) * E]),
                        (xsb[dk][:, tch * TCH:(tch + 1) * TCH]),
                        start=(dk == 0), stop=(dk == DK - 1),
                    )
                nc.scalar.copy(logitsT[:, tch * TCH:(tch + 1) * TCH], pg[:])
            for j in range(NC):
                pt = psG.tile([P, E], F32, tag="pstr")
                nc.tensor.transpose(
                    pt[:], logitsT[:, j * P:(j + 1) * P], ident[:])
                nc.scalar.copy(logits_tb[:, j * E:(j + 1) * E], pt[:])

            eL = gtmp.tile([P, NC * E], F32, tag="eL")
            nc.scalar.activation(eL[:], logits_tb[:], AF.Exp)
            e3 = eL[:].rearrange("p (c e) -> p c e", e=E)
            ssum = gtmp.tile([P, NC], F32, tag="ssum")
            nc.vector.reduce_sum(ssum[:], e3, axis=mybir.AxisListType.X)
            rs = gtmp.tile([P, NC], F32, tag="rs")
            nc.vector.reciprocal(rs[:], ssum[:])
            scores = gtmp.tile([P, NC * E], F32, tag="scores")
            s3 = scores[:].rearrange("p (c e) -> p c e", e=E)
            nc.vector.tensor_mul(
                s3, e3, rs[:, :, None].broadcast_to((P, NC, E)))
            biased = gtmp.tile([P, NC * E], F32, tag="biased")
            bi3 = biased[:].rearrange("p (c e) -> p c e", e=E)
            nc.vector.tensor_add(
                bi3, s3, gb_sb[:, None, :].broadcast_to((P, NC, E)))
            m1 = gtmp.tile([P, NC], F32, tag="m1")
            nc.vector.reduce_max(m1[:], bi3, axis=mybir.AxisListType.X)
            mask1 = gtmp.tile([P, NC * E], F32, tag="mask1")
            mk3 = mask1[:].rearrange("p (c e) -> p c e", e=E)
            nc.vector.tensor_tensor(
                mk3, bi3, m1[:, :, None].broadcast_to((P, NC, E)), OP.is_ge)
            biased2 = gtmp.tile([P, NC * E], F32, tag="biased2")
            b23 = biased2[:].rearrange("p (c e) -> p c e", e=E)
            nc.vector.scalar_tensor_tensor(
                out=b23, in0=mk3, scalar=-1e30, in1=bi3,
                op0=OP.mult, op1=OP.add)
            m2 = gtmp.tile([P, NC], F32, tag="m2")
            nc.vector.reduce_max(m2[:], b23, axis=mybir.AxisListType.X)
            mask2 = gtmp.tile([P, NC * E], F32, tag="mask2")
            mq3 = mask2[:].rearrange("p (c e) -> p c e", e=E)
            nc.vector.tensor_tensor(
                mq3, bi3, m2[:, :, None].broadcast_to((P, NC, E)), OP.is_ge)
            cwf = gtmp.tile([P, NC * E], F32, tag="cwf")
            cf3 = cwf[:].rearrange("p (c e) -> p c e", e=E)
            nc.vector.tensor_mul(cf3, s3, mq3)
            nc.vector.tensor_mul(
                cf3, cf3, sel_sb[:, None, :].broadcast_to((P, NC, E)))
            nc.vector.reduce_sum(cw[:], cf3, axis=mybir.AxisListType.X)

        # ---- main ----
        wA = ctx.enter_context(tc.tile_pool(name="wA", bufs=2))
        w2p = ctx.enter_context(tc.tile_pool(name="w2p", bufs=3))
        apool = ctx.enter_context(tc.tile_pool(name="apool", bufs=1))
        hpool = ctx.enter_context(tc.tile_pool(name="hpool", bufs=2))
        outp = ctx.enter_context(tc.tile_pool(name="outp", bufs=3))
        psA = ctx.enter_context(tc.tile_pool(name="psA", bufs=2, space="PSUM"))
        psB = ctx.enter_context(tc.tile_pool(name="psB", bufs=2, space="PSUM"))
        psS = ctx.enter_context(tc.tile_pool(name="psS", bufs=2, space="PSUM"))

        afc = FI + 1
        for th in range(NTH):
            tbase = th * TH
            atiles = []
            for fi in range(afc):
                at = apool.tile([P, TH], F32R, tag=f"a{fi}")
                atiles.append(at)
                if fi < FI:
                    wt = {}
                    for nm in ("w1g", "w1l", "w3g", "w3l"):
                        t = wA.tile([P, DK, P], F32R, tag=nm)
                        nc.sync.dma_start(t[:], aps[nm][fi])
                        wt[nm] = t
                    w_g1, w_l1 = wt["w1g"], wt["w1l"]
                    w_g3, w_l3 = wt["w3g"], wt["w3l"]
                else:
                    w_g1, w_l1 = ssw["sw1g"], ssw["sw1l"]
                    w_g3, w_l3 = ssw["sw3g"], ssw["sw3l"]
                bc1g = bcols["b1g"][:, fi:fi + 1]
                bc1l = bcols["b1l"][:, fi:fi + 1]
                bc3g = bcols["b3g"][:, fi:fi + 1]
                bc3l = bcols["b3l"][:, fi:fi + 1]

                for tt in range(TH // TCH):
                    tsl = slice(tt * TCH, (tt + 1) * TCH)
                    gsl = slice(tbase + tt * TCH, tbase + (tt + 1) * TCH)

                    def hpsum(wtile, ptag):
                        ps = psA.tile([P, TCH], F32, tag=ptag)
                        for dk in range(DK):
                            nc.tensor.matmul(
                                ps[:], (wtile[:, dk, :]),
                                (xsb[dk][:, gsl]),
                                start=(dk == 0), stop=(dk == DK - 1))
                        return ps

                    pg1 = hpsum(w_g1, "pA")
                    t1 = hpool.tile([P, TCH], F32, tag="tcp")
                    nc.scalar.activation(t1[:], pg1[:], AF.Identity, bias=bc1g)
                    pg3 = hpsum(w_g3, "pB")
                    hg = hpool.tile([P, TCH], F32, tag="hh")
                    nc.vector.scalar_tensor_tensor(
                        out=hg[:], in0=pg3[:], scalar=bc3g, in1=t1[:],
                        op0=OP.add, op1=OP.mult)
                    nc.vector.tensor_scalar_min(hg[:], hg[:], LIMIT)
                    gs = hpool.tile([P, TCH], F32, tag="gs")
                    nc.scalar.activation(gs[:], hg[:], AF.Silu, scale=ALPHA)

                    pl1 = hpsum(w_l1, "pA")
                    t2 = hpool.tile([P, TCH], F32, tag="tcp")
                    nc.scalar.activation(t2[:], pl1[:], AF.Identity, bias=bc1l)
                    pl3 = hpsum(w_l3, "pB")
                    hl = hpool.tile([P, TCH], F32, tag="hh")
                    nc.vector.scalar_tensor_tensor(
                        out=hl[:], in0=pl3[:], scalar=bc3l, in1=t2[:],
                        op0=OP.add, op1=OP.mult)
                    nc.vector.tensor_scalar(
                        out=hl[:], in0=hl[:], scalar1=LIMIT, scalar2=-LIMIT,
                        op0=OP.min, op1=OP.max)
                    nc.vector.tensor_scalar(
                        out=hl[:], in0=hl[:], scalar1=1.0 / ALPHA,
                        scalar2=1.0 / ALPHA, op0=OP.mult, op1=OP.add)
                    nc.vector.tensor_mul(atiles[fi][:, tsl], gs[:], hl[:])

            for tp in range(TH // P):
                j = th * (TH // P) + tp
                tsl = slice(tp * P, (tp + 1) * P)
                for dch in range(D // TCH):
                    dsl = slice(dch * TCH, (dch + 1) * TCH)
                    pB = psB.tile([P, TCH], F32, tag="pB2")
                    nc.tensor.matmul(pB[:], (ones[:]),
                                     (b2r_sb[0:1, dsl]),
                                     start=True, stop=False)
                    for fi in range(FI):
                        wt2 = w2p.tile([P, TCH], F32R, tag="w2t")
                        nc.sync.dma_start(
                            wt2[:], aps["w2T"][fi * P:(fi + 1) * P, dsl])
                        nc.tensor.matmul(
                            pB[:], (atiles[fi][:, tsl]), (wt2[:]),
                            start=False, stop=(fi == FI - 1))
                    pS = psS.tile([P, TCH], F32, tag="pS")
                    nc.tensor.matmul(pS[:], (ones[:]),
                                     (sb2r_sb[0:1, dsl]),
                                     start=True, stop=False)
                    nc.tensor.matmul(
                        pS[:], (atiles[FI][:, tsl]), (sw2T_sb[:, dsl]),
                        start=False, stop=True)
                    ot = outp.tile([P, TCH], F32, tag="ot")
                    nc.vector.tensor_scalar_mul(ot[:], pB[:], cw[:, j:j + 1])
                    nc.vector.tensor_add(ot[:], pS[:], ot[:])
                    nc.sync.dma_start(
                        aps["out"][tbase + tp * P:tbase + (tp + 1) * P, dsl],
                        ot[:])


def _prep_dense(x, gate_w, gate_b, w1, b1, w3, b3, w2, b2,
                sw1, sb1, sw3, sb3, sw2, sb2):
    f32 = np.float32
    xt = np.asarray(x, f32).reshape(T, D)
    xT = np.ascontiguousarray(xt.T)
    gwT = np.asarray(gate_w, f32).T
    gw_sb = np.ascontiguousarray(
        gwT.reshape(DK, P, E).transpose(1, 0, 2).reshape(P, DK * E))
    gb_bc = np.ascontiguousarray(
        np.broadcast_to(np.asarray(gate_b, f32), (P, E)))

    sw1 = np.asarray(sw1, f32)
    sw3 = np.asarray(sw3, f32)
    sw2T = np.asarray(sw2, f32).T
    sb1 = np.asarray(sb1, f32)
    sb3 = np.asarray(sb3, f32)
    sb2 = np.asarray(sb2, f32)

    def swarr(w_sl):
        return np.ascontiguousarray(
            w_sl.T.reshape(DK, P, P).transpose(1, 0, 2))

    def bcol2(b, sb_sl):
        return np.ascontiguousarray(
            np.concatenate([b.reshape(FI, P).T, sb_sl[:, None]], axis=1))

    in_maps = []
    for c in range(NCORES):
        sel = np.zeros((P, E), f32)
        sel[:, c] = 1.0
        w1c = np.asarray(w1[c], f32)
        w3c = np.asarray(w3[c], f32)
        b1c = np.asarray(b1[c], f32)
        b3c = np.asarray(b3[c], f32)
        fsl = slice(c * P, (c + 1) * P)
        m = {
            "xT": xT, "gw": gw_sb, "gb": gb_bc, "sel": sel,
            "w1g": _warr(w1c[0::2]), "w1l": _warr(w1c[1::2]),
            "w3g": _warr(w3c[0::2]), "w3l": _warr(w3c[1::2]),
            "b1g": bcol2(b1c[0::2], sb1[0::2][fsl]),
            "b1l": bcol2(b1c[1::2], sb1[1::2][fsl]),
            "b3g": bcol2(b3c[0::2], sb3[0::2][fsl]),
            "b3l": bcol2(b3c[1::2], sb3[1::2][fsl]),
            "w2T": np.ascontiguousarray(np.asarray(w2[c], f32).T),
            "b2r": np.asarray(b2[c], f32)[None, :],
            "sw1g": swarr(sw1[0::2][fsl]), "sw1l": swarr(sw1[1::2][fsl]),
            "sw3g": swarr(sw3[0::2][fsl]), "sw3l": swarr(sw3[1::2][fsl]),
            "sw2T": np.ascontiguousarray(sw2T[fsl]),
            "sb2r": (sb2 if c == 0 else np.zeros_like(sb2))[None, :],
            "ones": np.ones((1, P), f32),
        }
        in_maps.append(m)
    return in_maps


if __name__ == "__main__":
    rng = np.random.RandomState(0)
    sd = 1 / 32.0
    ins = {
        "x": rng.randn(2, 1024, 1024).astype(np.float32),
        "gate_w": (rng.randn(E, D) * sd).astype(np.float32),
        "gate_b": (rng.randn(E) * 0.01).astype(np.float32),
        "w1": (rng.randn(E, 2 * F, D) * sd).astype(np.float32),
        "b1": (rng.randn(E, 2 * F) * 0.01).astype(np.float32),
        "w3": (rng.randn(E, 2 * F, D) * sd).astype(np.float32),
        "b3": (rng.randn(E, 2 * F) * 0.01).astype(np.float32),
        "w2": (rng.randn(E, D, F) * sd).astype(np.float32),
        "b2": (rng.randn(E, D) * 0.01).astype(np.float32),
        "sw1": (rng.randn(2 * F, D) * sd).astype(np.float32),
        "sb1": (rng.randn(2 * F) * 0.01).astype(np.float32),
        "sw3": (rng.randn(2 * F, D) * sd).astype(np.float32),
        "sb3": (rng.randn(2 * F) * 0.01).astype(np.float32),
        "sw2": (rng.randn(D, F) * sd).astype(np.float32),
        "sb2": (rng.randn(D) * 0.01).astype(np.float32),
    }
    out = kernel(**ins)
    print("OK", out.shape, out.dtype, np.abs(out).mean())

